# revision 1
# baseline (speedup 1.0000x reference)
"""Trainium2 Bass kernel for nn_AtomAttention (B=2, N=2048, D=256, C=4, H=4).

Key algebraic property of the reference:

    weighted = einsum('bqkh,bvdh->bqdh', att, v)

has NO shared summation index between `att` and `v` (`k` and `v` are summed
independently), so it factorizes into

    weighted[b,q,d,h] = (sum_k att[b,q,k,h]) * (sum_v v[b,v,d,h])

and since `att` is a softmax over axis k, the first factor is exactly 1 for
every (b,q,h) — regardless of the attention scores, bias, mask or scaling.
Therefore the whole network reduces exactly (not approximately) to

    vsum[b,:]  = sum_n (atom_embed[b] @ Wv)[n, :]          # (B, D*H)
    gate       = sigmoid(atom_embed @ Wg + bg)             # (B, N, D*H)
    out        = (gate * vsum[:,None,:]) @ Wo + bo         # (B, N, D)

(verified: f64 full-reference vs f64 shortcut agree to ~7e-15 rel, including
with non-trivial masks; softmax normalizes over k no matter what the scores
are). molecular_matrix / Wq / Wk / W_bias / layernorm params / embedding_mask
cancel out of the forward value entirely, so the kernel never reads them.

Sharding: 8 cores, data-parallel over batch and sequence: core c handles
batch b=c//4, query rows [s*512,(s+1)*512) with s=c%4. Each core receives
the full transposed embed matrix of its batch (columns rolled so its own 512
rows come first) so it computes the batch-wide column-sum `vsum` locally (no
collectives), plus replicated projection weights.

Per-core device pipeline (all f32, exact):
  et (256,2048)  = E[b]^T rolled                     -> SBUF (2 x 128p tiles)
  esum (256,1)   = row-sum of et (free-axis reduce on the vector engine)
  gateT (128,512) x8 = sigmoid(Wg^T @ E_own^T + bg)  (PE matmul + ACT sigmoid
                       with per-partition bias, PSUM -> SBUF)
  vsumT (128,8)  = Wv^T-chunks @ esum                (PE, accumulated in PSUM)
  Wo'            = rows of Wo scaled by vsum         (per-partition
                       tensor_scalar on the vector engine)
  out (512,256)  = ones^T@bo (PSUM init) + sum_t gateT_t^T @ Wo'_t, then
                   copied out via ACT and DMA'd to HBM

The baseline variant of this pipeline (same math, output in natural
orientation, 69 matmuls) measured 68,571 ns HW exec time with absmax
relative error 1.5e-06 on 8 axon-tunneled trn2 NeuronCores. This version
computes the output transposed (outT = Wo'^T @ gateT), cutting the PE
stream to 48 matmuls, all with N=512 moving operands, and folding bo into
the per-partition bias of the PSUM->SBUF copy; CoreSim absmax rel 9.2e-07.
"""
import numpy as np
import concourse.bacc as bacc
import concourse.tile as tile
from concourse import mybir
from concourse.bass_utils import run_bass_kernel_spmd

B, N, D, H = 2, 2048, 256, 4
DH = D * H
NCORES = 8
CPB = NCORES // B
ROWS = N // CPB
P = 128
KC = D // P
TT = DH // P
MC = ROWS // P
F32 = mybir.dt.float32

def build_nc():
    nc = bacc.Bacc("TRN2", target_bir_lowering=False, debug=False, num_devices=NCORES)
    et_own = nc.dram_tensor("et_own", [D, ROWS], F32, kind="ExternalInput")
    et_rest = nc.dram_tensor("et_rest", [D, N - ROWS], F32, kind="ExternalInput")
    wg = nc.dram_tensor("wg", [D, DH], F32, kind="ExternalInput")
    wv = nc.dram_tensor("wv", [D, DH], F32, kind="ExternalInput")
    wo = nc.dram_tensor("wo", [DH, D], F32, kind="ExternalInput")
    bgt = nc.dram_tensor("bgt", [P, TT], F32, kind="ExternalInput")
    bot = nc.dram_tensor("bot", [P, D // P], F32, kind="ExternalInput")
    out = nc.dram_tensor("out", [D, ROWS], F32, kind="ExternalOutput")
    with tile.TileContext(nc) as tc:
        with (
            tc.tile_pool(name="sb", bufs=1) as sb,
            tc.tile_pool(name="osb", bufs=3) as osb,
            tc.tile_pool(name="ps_v", bufs=1, space="PSUM") as ps_v,
            tc.tile_pool(name="ps_g", bufs=3, space="PSUM") as ps_g,
            tc.tile_pool(name="ps_o", bufs=2, space="PSUM") as ps_o,
        ):
            eo_t = [sb.tile([P, ROWS], F32, name=f"eo{c}", tag=f"eo{c}") for c in range(KC)]
            er_t = [sb.tile([P, N - ROWS], F32, name=f"er{c}", tag=f"er{c}") for c in range(KC)]
            esp_t = [sb.tile([P, 2], F32, name=f"esp{c}", tag=f"esp{c}") for c in range(KC)]
            wg_t = [sb.tile([P, DH], F32, name=f"wg{c}", tag=f"wg{c}") for c in range(KC)]
            wv_t = [sb.tile([P, DH], F32, name=f"wv{c}", tag=f"wv{c}") for c in range(KC)]
            wo_t = [sb.tile([P, D], F32, name=f"wo{t}", tag=f"wo{t}") for t in range(TT)]
            wos_t = [sb.tile([P, D], F32, name=f"wos{t}", tag=f"wos{t}") for t in range(TT)]
            gt_t = [sb.tile([P, ROWS], F32, name=f"gt{t}", tag=f"gt{t}") for t in range(TT)]
            bgt_t = sb.tile([P, TT], F32, tag="bgt")
            bot_t = sb.tile([P, D // P], F32, tag="bot")
            es_t = [sb.tile([P, 1], F32, name=f"es{c}", tag=f"es{c}") for c in range(KC)]
            vs_sb = sb.tile([P, TT], F32, tag="vs")
            for c in range(KC):
                nc.sync.dma_start(eo_t[c][:], et_own[c*P:(c+1)*P, :])
            for c in range(KC):
                nc.sync.dma_start(er_t[c][:], et_rest[c*P:(c+1)*P, :])
            for c in range(KC):
                nc.sync.dma_start(wg_t[c][:], wg[c*P:(c+1)*P, :])
            for c in range(KC):
                nc.sync.dma_start(wv_t[c][:], wv[c*P:(c+1)*P, :])
            for t in range(TT):
                nc.sync.dma_start(wo_t[t][:], wo[t*P:(t+1)*P, :])
            nc.sync.dma_start(bgt_t[:], bgt[:])
            nc.sync.dma_start(bot_t[:], bot[:])
            for c in range(KC):
                nc.vector.reduce_sum(esp_t[c][:, 0:1], eo_t[c][:],
                                     axis=mybir.AxisListType.X)
                nc.vector.reduce_sum(esp_t[c][:, 1:2], er_t[c][:],
                                     axis=mybir.AxisListType.X)
                nc.vector.reduce_sum(es_t[c][:], esp_t[c][:],
                                     axis=mybir.AxisListType.X)
            for t in range(TT):
                g_ps = ps_g.tile([P, ROWS], F32)
                for c in range(KC):
                    nc.tensor.matmul(g_ps[:], wg_t[c][:, t*P:(t+1)*P], eo_t[c][:],
                                     start=(c==0), stop=(c==KC-1))
                nc.scalar.activation(gt_t[t][:], g_ps[:], mybir.ActivationFunctionType.Sigmoid,
                                     bias=bgt_t[:, t:t+1])
            vs_ps = ps_v.tile([P, TT], F32)
            for t in range(TT):
                for c in range(KC):
                    nc.tensor.matmul(vs_ps[:, t:t+1], wv_t[c][:, t*P:(t+1)*P], es_t[c][:],
                                     start=(c==0), stop=(c==KC-1))
            nc.vector.tensor_copy(vs_sb[:], vs_ps[:])
            for t in range(TT):
                nc.vector.tensor_scalar_mul(wos_t[t][:], wo_t[t][:], vs_sb[:, t:t+1])
            for m in range(D // P):
                o_ps = ps_o.tile([P, ROWS], F32)
                for t in range(TT):
                    nc.tensor.matmul(o_ps[:], wos_t[t][:, m*P:(m+1)*P], gt_t[t][:],
                                     start=(t==0), stop=(t==TT-1))
                o_sb = osb.tile([P, ROWS], F32, name="o", tag="o")
                nc.scalar.add(o_sb[:], o_ps[:], bot_t[:, m:m+1])
                nc.sync.dma_start(out[m*P:(m+1)*P, :], o_sb[:])
    nc.compile()
    return nc


_NC = None


def _get_nc():
    global _NC
    if _NC is None:
        _NC = build_nc()
    return _NC


def _make_in_maps(inputs):
    E = np.asarray(inputs["atom_embed"], dtype=np.float32)
    Wg = np.ascontiguousarray(np.asarray(inputs["Wg"], dtype=np.float32))
    Wv = np.ascontiguousarray(np.asarray(inputs["Wv"], dtype=np.float32))
    Wo = np.ascontiguousarray(np.asarray(inputs["Wo"], dtype=np.float32))
    bg = np.asarray(inputs["bg"], dtype=np.float32)
    bo = np.asarray(inputs["bo"], dtype=np.float32)

    bgt = np.ascontiguousarray(bg.reshape(TT, P).T)   # bgt[p, t] = bg[t*128+p]
    bot = np.ascontiguousarray(bo.reshape(D // P, P).T)  # bot[p, m] = bo[m*128+p]

    in_maps = []
    for c in range(NCORES):
        b, s = divmod(c, CPB)
        ET = E[b].T  # (D, N)
        # own 512 columns as one tensor; the rest (any order) only feeds
        # the order-independent esum reduction
        own = np.ascontiguousarray(ET[:, s * ROWS:(s + 1) * ROWS])
        rest = np.ascontiguousarray(
            np.concatenate([ET[:, (s + 1) * ROWS:], ET[:, :s * ROWS]], axis=1))
        in_maps.append({
            "et_own": own, "et_rest": rest,
            "wg": Wg, "wv": Wv, "wo": Wo,
            "bgt": bgt, "bot": bot,
        })
    return in_maps


def _run(inputs, trace=False):
    """Run on 8 NeuronCores; returns (full_output, BassKernelResults)."""
    in_maps = _make_in_maps(inputs)
    res = run_bass_kernel_spmd(_get_nc(), in_maps, list(range(NCORES)),
                               trace=trace)
    out = np.empty((B, N, D), dtype=np.float32)
    for c in range(NCORES):
        b, s = divmod(c, CPB)
        out[b, s * ROWS:(s + 1) * ROWS, :] = res.results[c]["out"].T
    return out, res


def kernel(**inputs) -> np.ndarray:
    out, _ = _run(inputs, trace=False)
    return out



# revision 4
# speedup vs baseline: 1.8667x; 1.8667x over previous
"""Trainium2 Bass kernel for nn_AtomAttention (B=2, N=2048, D=256, C=4, H=4).

Key algebraic property of the reference:

    weighted = einsum('bqkh,bvdh->bqdh', att, v)

has NO shared summation index between `att` and `v` (`k` and `v` are summed
independently), so it factorizes into

    weighted[b,q,d,h] = (sum_k att[b,q,k,h]) * (sum_v v[b,v,d,h])

and since `att` is a softmax over axis k, the first factor is exactly 1 for
every (b,q,h) — regardless of the attention scores, bias, mask or scaling.
Therefore the whole network reduces exactly (not approximately) to

    vsum[b,:]  = (sum_n atom_embed[b,n,:]) @ Wv              # (B, D*H)
    gate       = sigmoid(atom_embed @ Wg + bg)               # (B, N, D*H)
    out        = (gate * vsum[:,None,:]) @ Wo + bo           # (B, N, D)

molecular_matrix / Wq / Wk / W_bias / layernorm params / embedding_mask
cancel out of the forward value entirely, so the kernel never reads them.

Sharding: 8 cores, data-parallel over batch and sequence: core c handles
batch b=c//4, query rows [s*512,(s+1)*512) with s=c%4. Each core receives
the full E[b]^T (own 512 columns + the other 1536) so it computes the
batch-wide column-sum locally (no collectives), plus replicated weights.

This version runs everything in bf16 (fp32 PSUM accumulation) — tolerance
is 2e-2 absmax-rel and bf16 lands ~4e-3. That halves HBM traffic vs fp32
(2.56 MB/core) and runs the PE at full bf16 rate with FWL (fp32 matmul is
a double-pass at half rate with fast-weight-load disabled: ~5x slower).
DMA order is chosen so the PE never stalls: wg+eo first (gate matmuls
start at ~2.2us), then er (esum path), wv, wo. A burst of tiny warmup
matmuls during the initial DMA wait trips the PE HAM clock-gate to 2.4GHz
before the real matmuls arrive. Outputs are stored bf16 on the scalar
DMA ring and upcast on the host.
"""
import ml_dtypes
import numpy as np
import concourse.bacc as bacc
import concourse.tile as tile
from concourse import mybir
from concourse.bass_utils import run_bass_kernel_spmd

B, N, D, H = 2, 2048, 256, 4
DH = D * H
NCORES = 8
CPB = NCORES // B          # cores per batch
ROWS = N // CPB            # 512 query rows per core
REST = N - ROWS            # 1536
P = 128
KC = D // P                # 2 contraction blocks (d)
TT = DH // P               # 8 dh tiles
MC = D // P                # 2 output-d tiles
NCH = 3                    # er chunks of 512 columns
NWARM = 36
F32 = mybir.dt.float32
BF16 = mybir.dt.bfloat16
BF_NP = ml_dtypes.bfloat16


def build_nc():
    nc = bacc.Bacc("TRN2", target_bir_lowering=False, debug=False, num_devices=NCORES)
    eo = nc.dram_tensor("eo", [P, KC * ROWS], BF16, kind="ExternalInput")
    er = nc.dram_tensor("er", [P, KC * REST], BF16, kind="ExternalInput")
    wg = nc.dram_tensor("wg", [P, KC * DH], BF16, kind="ExternalInput")
    wv = nc.dram_tensor("wv", [P, KC * DH], BF16, kind="ExternalInput")
    wo = nc.dram_tensor("wo", [P, TT * D], BF16, kind="ExternalInput")
    bgt = nc.dram_tensor("bgt", [P, TT], F32, kind="ExternalInput")
    bot = nc.dram_tensor("bot", [P, MC], F32, kind="ExternalInput")
    out = nc.dram_tensor("out", [P, MC * ROWS], BF16, kind="ExternalOutput")
    with tile.TileContext(nc) as tc:
        with (
            tc.tile_pool(name="sb", bufs=1) as sb,
            tc.tile_pool(name="osb", bufs=2) as osb,
            tc.tile_pool(name="ps_w", bufs=1, space="PSUM") as ps_w,
            tc.tile_pool(name="ps_g", bufs=3, space="PSUM") as ps_g,
            tc.tile_pool(name="ps_v", bufs=1, space="PSUM") as ps_v,
            tc.tile_pool(name="ps_o", bufs=2, space="PSUM") as ps_o,
        ):
            warm = sb.tile([P, 16], BF16, tag="warm")
            eo_sb = sb.tile([P, KC * ROWS], BF16, tag="eo")
            er_sb = sb.tile([P, NCH * KC * 512], BF16, tag="er")
            wg_sb = sb.tile([P, KC * DH], BF16, tag="wg")
            wv_sb = sb.tile([P, KC * DH], BF16, tag="wv")
            wo_sb = sb.tile([P, TT * D], BF16, tag="wo")
            bgt_sb = sb.tile([P, TT], F32, tag="bgt")
            bot_sb = sb.tile([P, MC], F32, tag="bot")
            # esum partials: [c0: own, k0, k1, k2 | c1: own, k0, k1, k2]
            esp = sb.tile([P, 2 * 4], F32, tag="esp")
            esp_bf = sb.tile([P, 2 * 4], BF16, tag="espb")
            gt = [sb.tile([P, ROWS], BF16, name=f"gt{t}", tag=f"gt{t}") for t in range(TT)]
            vs_f = sb.tile([P, TT], F32, tag="vsf")
            wos = [sb.tile([P, D], BF16, name=f"wos{t}", tag=f"wos{t}") for t in range(TT)]

            # --- PE warmup: trip the HAM clock-gate to 2.4GHz during the DMA wait
            nc.gpsimd.memset(warm[:], 0.0)
            wps = ps_w.tile([16, 16], F32)
            for _ in range(NWARM):
                nc.tensor.matmul(wps[:], warm[:, 0:16], warm[:, 0:16],
                                 start=True, stop=True)

            # --- input DMAs, strict FIFO order on the sync HWDGE ring
            nc.sync.dma_start(bgt_sb[:], bgt[:])
            nc.sync.dma_start(bot_sb[:], bot[:])
            nc.sync.dma_start(wg_sb[:], wg[:])
            nc.sync.dma_start(eo_sb[:], eo[:])
            for k in range(NCH):
                w = KC * 512
                nc.sync.dma_start(er_sb[:, k * w:(k + 1) * w], er[:, k * w:(k + 1) * w])
            nc.sync.dma_start(wv_sb[:], wv[:])
            nc.sync.dma_start(wo_sb[:], wo[:])

            # --- esum partials (fp32) on DVE as data lands
            for c in range(KC):
                nc.vector.reduce_sum(esp[:, 4 * c:4 * c + 1],
                                     eo_sb[:, c * ROWS:(c + 1) * ROWS],
                                     axis=mybir.AxisListType.X)
            for k in range(NCH):
                for c in range(KC):
                    base = k * KC * 512 + c * 512
                    nc.vector.reduce_sum(esp[:, 4 * c + 1 + k:4 * c + 2 + k],
                                         er_sb[:, base:base + 512],
                                         axis=mybir.AxisListType.X)
            nc.vector.tensor_copy(esp_bf[:], esp[:])

            # --- gate: gT_t = sigmoid(Wg_t^T @ E_own^T + bg_t), bf16
            for t in range(TT):
                g_ps = ps_g.tile([P, ROWS], F32)
                for c in range(KC):
                    nc.tensor.matmul(g_ps[:],
                                     wg_sb[:, c * DH + t * P:c * DH + (t + 1) * P],
                                     eo_sb[:, c * ROWS:(c + 1) * ROWS],
                                     start=(c == 0), stop=(c == KC - 1))
                nc.scalar.activation(gt[t][:], g_ps[:],
                                     mybir.ActivationFunctionType.Sigmoid,
                                     bias=bgt_sb[:, t:t + 1])

            # --- vsum: per t, accumulate Wv_t^T @ (4 esum partial columns) over c
            vs_ps = ps_v.tile([P, TT, 4], F32)
            for t in range(TT):
                for c in range(KC):
                    nc.tensor.matmul(vs_ps[:, t, :],
                                     wv_sb[:, c * DH + t * P:c * DH + (t + 1) * P],
                                     esp_bf[:, 4 * c:4 * (c + 1)],
                                     start=(c == 0), stop=(c == KC - 1))
            nc.vector.reduce_sum(vs_f[:], vs_ps[:], axis=mybir.AxisListType.X)

            # --- wos_t = vsum_t * Wo_t rows, split across DVE and ACT
            for t in range(TT):
                if t % 2 == 0:
                    nc.vector.tensor_scalar_mul(wos[t][:], wo_sb[:, t * D:(t + 1) * D],
                                                vs_f[:, t:t + 1])
                else:
                    nc.scalar.mul(wos[t][:], wo_sb[:, t * D:(t + 1) * D],
                                  vs_f[:, t:t + 1])

            # --- out: outT_m = sum_t wos_t[:,m]^T @ gT_t  (+ bo), bf16 store
            for m in range(MC):
                o_ps = ps_o.tile([P, ROWS], F32)
                for t in range(TT):
                    nc.tensor.matmul(o_ps[:], wos[t][:, m * P:(m + 1) * P], gt[t][:],
                                     start=(t == 0), stop=(t == TT - 1))
                o_sb = osb.tile([P, ROWS], BF16, name="o", tag="o")
                if m % 2 == 0:
                    nc.vector.tensor_scalar_add(o_sb[:], o_ps[:], bot_sb[:, m:m + 1])
                else:
                    nc.scalar.add(o_sb[:], o_ps[:], bot_sb[:, m:m + 1])
                nc.scalar.dma_start(out[:, m * ROWS:(m + 1) * ROWS], o_sb[:])
    nc.compile()
    return nc


_NC = None


def _get_nc():
    global _NC
    if _NC is None:
        _NC = build_nc()
    return _NC


def _make_in_maps(inputs):
    E = np.asarray(inputs["atom_embed"], dtype=np.float32)
    Wg = np.asarray(inputs["Wg"], dtype=np.float32)
    Wv = np.asarray(inputs["Wv"], dtype=np.float32)
    Wo = np.asarray(inputs["Wo"], dtype=np.float32)
    bg = np.asarray(inputs["bg"], dtype=np.float32)
    bo = np.asarray(inputs["bo"], dtype=np.float32)

    # c-block-major packings (partition dim = 128)
    wg_np = np.concatenate([Wg[c * P:(c + 1) * P, :] for c in range(KC)], axis=1)
    wv_np = np.concatenate([Wv[c * P:(c + 1) * P, :] for c in range(KC)], axis=1)
    wo_np = np.concatenate([Wo[t * P:(t + 1) * P, :] for t in range(TT)], axis=1)
    wg_np = np.ascontiguousarray(wg_np).astype(BF_NP)
    wv_np = np.ascontiguousarray(wv_np).astype(BF_NP)
    wo_np = np.ascontiguousarray(wo_np).astype(BF_NP)
    bgt = np.ascontiguousarray(bg.reshape(TT, P).T)     # bgt[p, t] = bg[t*128+p]
    bot = np.ascontiguousarray(bo.reshape(MC, P).T)     # bot[p, m] = bo[m*128+p]

    in_maps = []
    for core in range(NCORES):
        b, s = divmod(core, CPB)
        ET = E[b].T.astype(BF_NP)  # (D, N) bf16
        own = ET[:, s * ROWS:(s + 1) * ROWS]
        rest = np.concatenate([ET[:, (s + 1) * ROWS:], ET[:, :s * ROWS]], axis=1)
        eo_np = np.concatenate([own[c * P:(c + 1) * P, :] for c in range(KC)], axis=1)
        er_np = np.concatenate(
            [rest[c * P:(c + 1) * P, k * 512:(k + 1) * 512]
             for k in range(NCH) for c in range(KC)], axis=1)
        in_maps.append({
            "eo": np.ascontiguousarray(eo_np),
            "er": np.ascontiguousarray(er_np),
            "wg": wg_np, "wv": wv_np, "wo": wo_np,
            "bgt": bgt, "bot": bot,
        })
    return in_maps


def _run(inputs, trace=False):
    """Run on 8 NeuronCores; returns (full_output, BassKernelResults)."""
    in_maps = _make_in_maps(inputs)
    res = run_bass_kernel_spmd(_get_nc(), in_maps, list(range(NCORES)),
                               trace=trace)
    out = np.empty((B, N, D), dtype=np.float32)
    for core in range(NCORES):
        b, s = divmod(core, CPB)
        o = res.results[core]["out"]  # (128, 2*512) bf16, m-major
        oT = np.concatenate([o[:, m * ROWS:(m + 1) * ROWS] for m in range(MC)],
                            axis=0).astype(np.float32)  # (256, 512)
        out[b, s * ROWS:(s + 1) * ROWS, :] = oT.T
    return out, res


def kernel(**inputs) -> np.ndarray:
    out, _ = _run(inputs, trace=False)
    return out


# revision 6
# speedup vs baseline: 2.0489x; 1.0976x over previous
"""Trainium2 Bass kernel for nn_AtomAttention (B=2, N=2048, D=256, C=4, H=4).

Key algebraic property of the reference:

    weighted = einsum('bqkh,bvdh->bqdh', att, v)

has NO shared summation index between `att` and `v` (`k` and `v` are summed
independently), so it factorizes into

    weighted[b,q,d,h] = (sum_k att[b,q,k,h]) * (sum_v v[b,v,d,h])

and since `att` is a softmax over axis k, the first factor is exactly 1 for
every (b,q,h) — regardless of the attention scores, bias, mask or scaling.
Therefore the whole network reduces exactly (not approximately) to

    vsum[b,:]  = (sum_n atom_embed[b,n,:]) @ Wv              # (B, D*H)
    gate       = sigmoid(atom_embed @ Wg + bg)               # (B, N, D*H)
    out        = (gate * vsum[:,None,:]) @ Wo + bo           # (B, N, D)

molecular_matrix / Wq / Wk / W_bias / layernorm params / embedding_mask
cancel out of the forward value entirely, so the kernel never reads them.

Sharding: 8 cores, data-parallel over batch and sequence: core c handles
batch b=c//4, query rows [s*512,(s+1)*512); each core gets the full E[b]^T
(own 512 columns first) so the batch column-sum is local (no collectives),
plus replicated weights.

Everything runs in bf16 (fp32 PSUM/partials) — tolerance is 2e-2 and this
lands ~5e-3 — halving HBM bytes and running the PE at full bf16/FWL rate
(fp32 matmul is a half-rate double-pass with fast-weight-load disabled).

Pipeline notes (v2): input DMAs are a few wide transfers in strict FIFO
order on the sync HWDGE ring ([wg|eo], er half 1, er half 2, wv, wo) so the
gate path starts as early as possible; the tiny bias tensor rides the
scalar ring. A short burst of warmup matmuls trips the PE HAM clock-gate
to 2.4GHz during the DMA wait. The two output m-tiles accumulate
interleaved per t (separate PSUM banks) so the last gate tile gates only
two matmuls. Output is stored bf16 on the scalar ring, upcast on host.
"""
import ml_dtypes
import numpy as np
import concourse.bacc as bacc
import concourse.tile as tile
from concourse import mybir
from concourse.bass_utils import run_bass_kernel_spmd

B, N, D, H = 2, 2048, 256, 4
DH = D * H
NCORES = 8
CPB = NCORES // B          # cores per batch
ROWS = N // CPB            # 512 query rows per core
REST = N - ROWS            # 1536
HREST = REST // 2          # 768 columns per er half
P = 128
KC = D // P                # 2 contraction blocks (d)
TT = DH // P               # 8 dh tiles
MC = D // P                # 2 output-d tiles
NWARM = 14
F32 = mybir.dt.float32
BF16 = mybir.dt.bfloat16
BF_NP = ml_dtypes.bfloat16

W1 = KC * (DH + ROWS)      # packed [wg | eo] columns: 3072


def build_nc():
    nc = bacc.Bacc("TRN2", target_bir_lowering=False, debug=False, num_devices=NCORES)
    w1 = nc.dram_tensor("w1", [P, W1], BF16, kind="ExternalInput")      # [wg c0,c1 | eo c0,c1]
    er = nc.dram_tensor("er", [P, KC * REST], BF16, kind="ExternalInput")  # [h][c][768]
    wv = nc.dram_tensor("wv", [P, KC * DH], BF16, kind="ExternalInput")
    wo = nc.dram_tensor("wo", [P, TT * D], BF16, kind="ExternalInput")
    bias = nc.dram_tensor("bias", [P, TT + MC], F32, kind="ExternalInput")
    out = nc.dram_tensor("out", [P, MC * ROWS], BF16, kind="ExternalOutput")
    EO0 = KC * DH  # eo offset inside w1
    with tile.TileContext(nc) as tc:
        with (
            tc.tile_pool(name="sb", bufs=1) as sb,
            tc.tile_pool(name="osb", bufs=2) as osb,
            tc.tile_pool(name="ps_w", bufs=1, space="PSUM") as ps_w,
            tc.tile_pool(name="ps_g", bufs=3, space="PSUM") as ps_g,
            tc.tile_pool(name="ps_v", bufs=1, space="PSUM") as ps_v,
            tc.tile_pool(name="ps_o", bufs=1, space="PSUM") as ps_o,
        ):
            warm = sb.tile([P, 64], BF16, tag="warm")
            w1_sb = sb.tile([P, W1], BF16, tag="w1")
            er_sb = sb.tile([P, KC * REST], BF16, tag="er")
            wv_sb = sb.tile([P, KC * DH], BF16, tag="wv")
            wo_sb = sb.tile([P, TT * D], BF16, tag="wo")
            bias_sb = sb.tile([P, TT + MC], F32, tag="bias")
            # esum partials: [c: own, h0, h1]
            esp = sb.tile([P, KC, 3], F32, tag="esp")
            esp_bf = sb.tile([P, KC, 3], BF16, tag="espb")
            gt = [sb.tile([P, ROWS], BF16, name=f"gt{t}", tag=f"gt{t}") for t in range(TT)]
            vs_f = sb.tile([P, TT], F32, tag="vsf")
            wos = [sb.tile([P, D], BF16, name=f"wos{t}", tag=f"wos{t}") for t in range(TT)]

            def wg_ap(c, t):
                return w1_sb[:, c * DH + t * P:c * DH + (t + 1) * P]

            def eo_ap(c):
                return w1_sb[:, EO0 + c * ROWS:EO0 + (c + 1) * ROWS]

            # --- PE warmup: trip the HAM clock-gate to 2.4GHz during the DMA wait
            nc.vector.memset(warm[:], 0.0)
            wps = ps_w.tile([64, 64], F32)
            for _ in range(NWARM):
                nc.tensor.matmul(wps[:], warm[:, 0:64], warm[:, 0:64],
                                 start=True, stop=True)

            # --- input DMAs: bias on the scalar ring; the rest strict FIFO on sync
            nc.scalar.dma_start(bias_sb[:], bias[:])
            nc.sync.dma_start(w1_sb[:], w1[:])
            for h in range(2):
                w = KC * HREST
                nc.sync.dma_start(er_sb[:, h * w:(h + 1) * w], er[:, h * w:(h + 1) * w])
            nc.sync.dma_start(wv_sb[:], wv[:])
            nc.sync.dma_start(wo_sb[:], wo[:])

            # --- gate: gT_t = sigmoid(Wg_t^T @ E_own^T + bg_t), bf16
            for t in range(TT):
                g_ps = ps_g.tile([P, ROWS], F32)
                for c in range(KC):
                    nc.tensor.matmul(g_ps[:], wg_ap(c, t), eo_ap(c),
                                     start=(c == 0), stop=(c == KC - 1))
                nc.scalar.activation(gt[t][:], g_ps[:],
                                     mybir.ActivationFunctionType.Sigmoid,
                                     bias=bias_sb[:, t:t + 1])

            # --- esum partials (fp32) on DVE as data lands
            for c in range(KC):
                nc.vector.reduce_sum(esp[:, c, 0:1], eo_ap(c), axis=mybir.AxisListType.X)
            for h in range(2):
                for c in range(KC):
                    base = h * KC * HREST + c * HREST
                    nc.vector.reduce_sum(esp[:, c, 1 + h:2 + h],
                                         er_sb[:, base:base + HREST],
                                         axis=mybir.AxisListType.X)
            nc.vector.tensor_copy(esp_bf[:], esp[:])

            # --- vsum: per t, accumulate Wv_t^T @ (3 esum partial columns) over c
            vs_ps = ps_v.tile([P, TT, 3], F32)
            for t in range(TT):
                for c in range(KC):
                    nc.tensor.matmul(vs_ps[:, t, :],
                                     wv_sb[:, c * DH + t * P:c * DH + (t + 1) * P],
                                     esp_bf[:, c, :],
                                     start=(c == 0), stop=(c == KC - 1))
            nc.vector.reduce_sum(vs_f[:], vs_ps[:], axis=mybir.AxisListType.X)

            # --- wos_t = vsum_t * Wo_t rows (DVE)
            for t in range(TT):
                nc.vector.tensor_scalar_mul(wos[t][:], wo_sb[:, t * D:(t + 1) * D],
                                            vs_f[:, t:t + 1])

            # --- out: outT_m = sum_t wos_t[:,m]^T @ gT_t (+ bo); m groups interleaved
            o_ps = [ps_o.tile([P, ROWS], F32, name=f"ops{m}") for m in range(MC)]
            for t in range(TT):
                for m in range(MC):
                    nc.tensor.matmul(o_ps[m][:], wos[t][:, m * P:(m + 1) * P], gt[t][:],
                                     start=(t == 0), stop=(t == TT - 1))
            for m in range(MC):
                o_sb = osb.tile([P, ROWS], BF16, name="o", tag="o")
                if m % 2 == 0:
                    nc.vector.tensor_scalar_add(o_sb[:], o_ps[m][:],
                                                bias_sb[:, TT + m:TT + m + 1])
                else:
                    nc.scalar.add(o_sb[:], o_ps[m][:], bias_sb[:, TT + m:TT + m + 1])
                nc.scalar.dma_start(out[:, m * ROWS:(m + 1) * ROWS], o_sb[:])
    nc.compile()
    return nc


_NC = None


def _get_nc():
    global _NC
    if _NC is None:
        _NC = build_nc()
    return _NC


def _make_in_maps(inputs):
    E = np.asarray(inputs["atom_embed"], dtype=np.float32)
    Wg = np.asarray(inputs["Wg"], dtype=np.float32)
    Wv = np.asarray(inputs["Wv"], dtype=np.float32)
    Wo = np.asarray(inputs["Wo"], dtype=np.float32)
    bg = np.asarray(inputs["bg"], dtype=np.float32)
    bo = np.asarray(inputs["bo"], dtype=np.float32)

    # c-block-major packings (partition dim = 128)
    wg_np = np.concatenate([Wg[c * P:(c + 1) * P, :] for c in range(KC)], axis=1)
    wv_np = np.concatenate([Wv[c * P:(c + 1) * P, :] for c in range(KC)], axis=1)
    wo_np = np.concatenate([Wo[t * P:(t + 1) * P, :] for t in range(TT)], axis=1)
    wv_np = np.ascontiguousarray(wv_np).astype(BF_NP)
    wo_np = np.ascontiguousarray(wo_np).astype(BF_NP)
    bias_np = np.ascontiguousarray(np.concatenate(
        [bg.reshape(TT, P).T, bo.reshape(MC, P).T], axis=1))  # (128, 10) f32

    in_maps = []
    for core in range(NCORES):
        b, s = divmod(core, CPB)
        ET = E[b].T.astype(BF_NP)  # (D, N) bf16
        own = ET[:, s * ROWS:(s + 1) * ROWS]
        rest = np.concatenate([ET[:, (s + 1) * ROWS:], ET[:, :s * ROWS]], axis=1)
        eo_np = np.concatenate([own[c * P:(c + 1) * P, :] for c in range(KC)], axis=1)
        w1_np = np.concatenate([wg_np.astype(BF_NP), eo_np], axis=1)
        er_np = np.concatenate(
            [rest[c * P:(c + 1) * P, h * HREST:(h + 1) * HREST]
             for h in range(2) for c in range(KC)], axis=1)
        in_maps.append({
            "w1": np.ascontiguousarray(w1_np),
            "er": np.ascontiguousarray(er_np),
            "wv": wv_np, "wo": wo_np, "bias": bias_np,
        })
    return in_maps


def _run(inputs, trace=False):
    """Run on 8 NeuronCores; returns (full_output, BassKernelResults)."""
    in_maps = _make_in_maps(inputs)
    res = run_bass_kernel_spmd(_get_nc(), in_maps, list(range(NCORES)),
                               trace=trace)
    out = np.empty((B, N, D), dtype=np.float32)
    for core in range(NCORES):
        b, s = divmod(core, CPB)
        o = res.results[core]["out"]  # (128, 2*512) bf16, m-major
        oT = np.concatenate([o[:, m * ROWS:(m + 1) * ROWS] for m in range(MC)],
                            axis=0).astype(np.float32)  # (256, 512)
        out[b, s * ROWS:(s + 1) * ROWS, :] = oT.T
    return out, res


def kernel(**inputs) -> np.ndarray:
    out, _ = _run(inputs, trace=False)
    return out


# revision 9
# speedup vs baseline: 2.1170x; 1.0332x over previous
"""Trainium2 Bass kernel for nn_AtomAttention (B=2, N=2048, D=256, C=4, H=4).

Key algebraic property of the reference:

    weighted = einsum('bqkh,bvdh->bqdh', att, v)

has NO shared summation index between `att` and `v` (`k` and `v` are summed
independently), so it factorizes into

    weighted[b,q,d,h] = (sum_k att[b,q,k,h]) * (sum_v v[b,v,d,h])

and since `att` is a softmax over axis k, the first factor is exactly 1 for
every (b,q,h) — regardless of the attention scores, bias, mask or scaling.
Therefore the whole network reduces exactly (not approximately) to

    vsum[b,:]  = (sum_n atom_embed[b,n,:]) @ Wv              # (B, D*H)
    gate       = sigmoid(atom_embed @ Wg + bg)               # (B, N, D*H)
    out        = (gate * vsum[:,None,:]) @ Wo + bo           # (B, N, D)

molecular_matrix / Wq / Wk / W_bias / layernorm params / embedding_mask
cancel out of the forward value entirely, so the kernel never reads them.

Sharding: 8 cores, data-parallel over batch and sequence: core c handles
batch b=c//4, query rows [s*512,(s+1)*512); each core gets the full E[b]^T
(own 512 columns first) so the batch column-sum is local (no collectives),
plus replicated weights.

Everything runs in bf16 (fp32 PSUM/partials) — tolerance is 2e-2 and this
lands ~6e-3 — halving HBM bytes and running the PE at full bf16/FWL rate.

Scheduling notes (v4, from trace analysis):
- The scalar HWDGE ring clears its engine preamble ~3us before the sync
  ring, so the gate operands [wg|eo] ride the scalar ring (first transfer
  ~5us) while er/wv/wo stream on the sync ring in parallel.
- ACT keeps a single table set (Sigmoid); output bias-adds run on DVE —
  a scalar.add would trigger a ~2.7us ACT table switch. (An ACTIVATE over
  a 2-bank PSUM region crashes the device — sigmoids stay per-t, N=512.)
- The ACT sigmoid stream (8 x ~720ns) is the critical pipe: the out MMs
  run as interleaved (m0,m1) pairs per t, locksteping the ACT stream, so
  only ~2 matmuls trail the last sigmoid.
- vsum -> vs -> wos pipeline per t-tile so wos never gates the out MMs.
"""
import ml_dtypes
import numpy as np
import concourse.bacc as bacc
import concourse.tile as tile
from concourse import mybir
from concourse.bass_utils import run_bass_kernel_spmd

B, N, D, H = 2, 2048, 256, 4
DH = D * H
NCORES = 8
CPB = NCORES // B          # cores per batch
ROWS = N // CPB            # 512 query rows per core
REST = N - ROWS            # 1536
HREST = REST // 2          # 768 columns per er half
P = 128
KC = D // P                # 2 contraction blocks (d)
TT = DH // P               # 8 dh tiles
MC = D // P                # 2 output-d tiles
NWARM = 20
F32 = mybir.dt.float32
BF16 = mybir.dt.bfloat16
BF_NP = ml_dtypes.bfloat16

W1 = KC * (DH + ROWS)      # packed [wg | eo] columns: 3072


def build_nc():
    nc = bacc.Bacc("TRN2", target_bir_lowering=False, debug=False, num_devices=NCORES)
    w1 = nc.dram_tensor("w1", [P, W1], BF16, kind="ExternalInput")      # [wg c0,c1 | eo c0,c1]
    er = nc.dram_tensor("er", [P, KC * REST], BF16, kind="ExternalInput")  # [h][c][768]
    wv = nc.dram_tensor("wv", [P, KC * DH], BF16, kind="ExternalInput")
    wo = nc.dram_tensor("wo", [P, TT * D], BF16, kind="ExternalInput")
    bias = nc.dram_tensor("bias", [P, TT + MC], F32, kind="ExternalInput")
    out = nc.dram_tensor("out", [P, MC * ROWS], BF16, kind="ExternalOutput")
    EO0 = KC * DH  # eo offset inside w1
    with tile.TileContext(nc) as tc:
        with (
            tc.tile_pool(name="sb", bufs=1) as sb,
            tc.tile_pool(name="osb", bufs=2) as osb,
            tc.tile_pool(name="ps_w", bufs=1, space="PSUM") as ps_w,
            tc.tile_pool(name="ps_g", bufs=4, space="PSUM") as ps_g,
            tc.tile_pool(name="ps_v", bufs=1, space="PSUM") as ps_v,
            tc.tile_pool(name="ps_o", bufs=1, space="PSUM") as ps_o,
        ):
            warm = sb.tile([P, 16], BF16, tag="warm")
            w1_sb = sb.tile([P, W1], BF16, tag="w1")
            er_sb = sb.tile([P, KC * REST], BF16, tag="er")
            wv_sb = sb.tile([P, KC * DH], BF16, tag="wv")
            wo_sb = sb.tile([P, TT * D], BF16, tag="wo")
            bias_sb = sb.tile([P, TT + MC], F32, tag="bias")
            # esum partials: [c: own, h0, h1]
            esp = sb.tile([P, KC, 3], F32, tag="esp")
            esp_bf = sb.tile([P, KC, 3], BF16, tag="espb")
            gt = [sb.tile([P, ROWS], BF16, name=f"gt{t}", tag=f"gt{t}")
                  for t in range(TT)]
            vs_f = sb.tile([P, TT], F32, tag="vsf")
            wos = [sb.tile([P, D], BF16, name=f"wos{t}", tag=f"wos{t}") for t in range(TT)]

            def wg_ap(c, t):
                return w1_sb[:, c * DH + t * P:c * DH + (t + 1) * P]

            def eo_ap(c):
                return w1_sb[:, EO0 + c * ROWS:EO0 + (c + 1) * ROWS]

            # --- tiny PE warmup: start the HAM busy-window before the real MMs
            nc.vector.memset(warm[:], 0.0)
            wps = ps_w.tile([16, 16], F32)
            for _ in range(NWARM):
                nc.tensor.matmul(wps[:], warm[:], warm[:], start=True, stop=True)

            # --- input DMAs: gate operands on the (earlier) scalar ring,
            #     esum/vsum/out operands FIFO on the sync ring
            nc.scalar.dma_start(w1_sb[:], w1[:])
            nc.scalar.dma_start(bias_sb[:], bias[:])
            for h in range(2):
                w = KC * HREST
                nc.sync.dma_start(er_sb[:, h * w:(h + 1) * w], er[:, h * w:(h + 1) * w])
            nc.sync.dma_start(wv_sb[:], wv[:])
            nc.sync.dma_start(wo_sb[:], wo[:])

            # --- gate: gT_t = sigmoid(Wg_t^T @ E_own^T + bg_t), bf16
            for t in range(TT):
                g_ps = ps_g.tile([P, ROWS], F32)
                for c in range(KC):
                    nc.tensor.matmul(g_ps[:], wg_ap(c, t), eo_ap(c),
                                     start=(c == 0), stop=(c == KC - 1))
                nc.scalar.activation(gt[t][:], g_ps[:],
                                     mybir.ActivationFunctionType.Sigmoid,
                                     bias=bias_sb[:, t:t + 1])

            # --- esum partials (fp32) on DVE as er halves land
            for c in range(KC):
                nc.vector.reduce_sum(esp[:, c, 0:1], eo_ap(c), axis=mybir.AxisListType.X)
            for h in range(2):
                for c in range(KC):
                    base = h * KC * HREST + c * HREST
                    nc.vector.reduce_sum(esp[:, c, 1 + h:2 + h],
                                         er_sb[:, base:base + HREST],
                                         axis=mybir.AxisListType.X)
            nc.vector.tensor_copy(esp_bf[:], esp[:])

            # --- vsum -> vs -> wos pipelined per t
            vs_ps = ps_v.tile([P, TT, 3], F32)
            for t in range(TT):
                for c in range(KC):
                    nc.tensor.matmul(vs_ps[:, t, :],
                                     wv_sb[:, c * DH + t * P:c * DH + (t + 1) * P],
                                     esp_bf[:, c, :],
                                     start=(c == 0), stop=(c == KC - 1))
                nc.vector.reduce_sum(vs_f[:, t:t + 1], vs_ps[:, t, :],
                                     axis=mybir.AxisListType.X)
                nc.vector.tensor_scalar_mul(wos[t][:], wo_sb[:, t * D:(t + 1) * D],
                                            vs_f[:, t:t + 1])

            # --- out: outT_m = sum_t wos_t[:,m]^T @ gT_t (+ bo); m groups
            # interleaved per t so only 2 MMs trail the last sigmoid
            o_ps = [ps_o.tile([P, ROWS], F32, name=f"ops{m}") for m in range(MC)]
            for t in range(TT):
                for m in range(MC):
                    nc.tensor.matmul(o_ps[m][:], wos[t][:, m * P:(m + 1) * P],
                                     gt[t][:], start=(t == 0), stop=(t == TT - 1))
            for m in range(MC):
                o_sb = osb.tile([P, ROWS], BF16, name="o", tag="o")
                nc.vector.tensor_scalar_add(o_sb[:], o_ps[m][:],
                                            bias_sb[:, TT + m:TT + m + 1])
                nc.scalar.dma_start(out[:, m * ROWS:(m + 1) * ROWS], o_sb[:])
    nc.compile()
    return nc


_NC = None


def _get_nc():
    global _NC
    if _NC is None:
        _NC = build_nc()
    return _NC


def _make_in_maps(inputs):
    E = np.asarray(inputs["atom_embed"], dtype=np.float32)
    Wg = np.asarray(inputs["Wg"], dtype=np.float32)
    Wv = np.asarray(inputs["Wv"], dtype=np.float32)
    Wo = np.asarray(inputs["Wo"], dtype=np.float32)
    bg = np.asarray(inputs["bg"], dtype=np.float32)
    bo = np.asarray(inputs["bo"], dtype=np.float32)

    # c-block-major packings (partition dim = 128)
    wg_np = np.concatenate([Wg[c * P:(c + 1) * P, :] for c in range(KC)], axis=1)
    wv_np = np.concatenate([Wv[c * P:(c + 1) * P, :] for c in range(KC)], axis=1)
    wo_np = np.concatenate([Wo[t * P:(t + 1) * P, :] for t in range(TT)], axis=1)
    wv_np = np.ascontiguousarray(wv_np).astype(BF_NP)
    wo_np = np.ascontiguousarray(wo_np).astype(BF_NP)
    bias_np = np.ascontiguousarray(np.concatenate(
        [bg.reshape(TT, P).T, bo.reshape(MC, P).T], axis=1))  # (128, 10) f32

    in_maps = []
    for core in range(NCORES):
        b, s = divmod(core, CPB)
        ET = E[b].T.astype(BF_NP)  # (D, N) bf16
        own = ET[:, s * ROWS:(s + 1) * ROWS]
        rest = np.concatenate([ET[:, (s + 1) * ROWS:], ET[:, :s * ROWS]], axis=1)
        eo_np = np.concatenate([own[c * P:(c + 1) * P, :] for c in range(KC)], axis=1)
        w1_np = np.concatenate([wg_np.astype(BF_NP), eo_np], axis=1)
        er_np = np.concatenate(
            [rest[c * P:(c + 1) * P, h * HREST:(h + 1) * HREST]
             for h in range(2) for c in range(KC)], axis=1)
        in_maps.append({
            "w1": np.ascontiguousarray(w1_np),
            "er": np.ascontiguousarray(er_np),
            "wv": wv_np, "wo": wo_np, "bias": bias_np,
        })
    return in_maps


def _run(inputs, trace=False):
    """Run on 8 NeuronCores; returns (full_output, BassKernelResults)."""
    in_maps = _make_in_maps(inputs)
    res = run_bass_kernel_spmd(_get_nc(), in_maps, list(range(NCORES)),
                               trace=trace)
    out = np.empty((B, N, D), dtype=np.float32)
    for core in range(NCORES):
        b, s = divmod(core, CPB)
        o = res.results[core]["out"]  # (128, 2*512) bf16, m-major
        oT = np.concatenate([o[:, m * ROWS:(m + 1) * ROWS] for m in range(MC)],
                            axis=0).astype(np.float32)  # (256, 512)
        out[b, s * ROWS:(s + 1) * ROWS, :] = oT.T
    return out, res


def kernel(**inputs) -> np.ndarray:
    out, _ = _run(inputs, trace=False)
    return out
